# revision 15
# baseline (speedup 1.0000x reference)
"""Trainium2 Bass kernel for the neural-backflow problem.

Problem (hardcoded shapes): rs (4096, 3) f32 in a periodic box L=10.
For every electron pair (i, j): minimum-image displacement d_ij, distance
r_ij, force f_ij = MLP_spin(r_ij) (1->32->1 swish MLP with compact-support
decay; "same" weights for same-spin pairs, "diff" for cross-spin), output
rs + sum_j f_ij * d_ij.

Key algebraic reduction used here: with z_k = decay*w1_k + b1_k,
  force = decay^2 * sum_k (w1_k*wo_k) * sigmoid(z_k) + bo*decay
which is a smooth scalar function P(decay) on decay in (0, 1].  We fit a
degree-10 polynomial (Chebyshev fit, monomial coeffs, P(0)=0 forced) to P at
kernel-call time from the actual weight values, so the device program is
input-independent: the MLP collapses to a Horner chain of
scalar_tensor_tensor ops on the Vector engine.

decay itself is computed exactly (not approximated):
  m   = ((rs_j - rs_i + 15) mod 10) - 5          (= -minimum-image disp)
  r2  = m_x^2 + m_y^2 + m_z^2                     (matches sqrt(r2+1e-15)^2)
  g   = clamp(1 - 0.04*(r2 + 1e-15), >= 1-(1-1e-5)^2)   (= 1 - xn^2)
  decay = exp(1 - 1/g),  with 1/g = exp(-ln g) on the ACT engine
  (ScalarE Reciprocal is banned; Ln/Exp/Square/Copy share one ACT table set)

Sharding: rows of the pair grid across 8 cores (512 rows each); rs is
replicated (pre-broadcast across 128 partitions host-side for the j-axis
tiles).  Row-sums are local per core; outputs are concatenated.
"""

import numpy as np

import concourse.bass as bass
import concourse.mybir as mybir
from concourse.tile import TileContext
from concourse.bass_utils import run_bass_kernel_spmd

L = 10.0
N = 4096
N_UP = 2048
NCORES = 8
ROWS = N // NCORES          # 512 rows per core
JT = 512                    # j-tile width
NJT = N // JT               # 8 j-tiles
NIB = ROWS // 128           # 4 i-blocks of 128 rows per core
DEG = 10                    # polynomial degree
GMIN = float(np.float32(1.0) - np.float32((1.0 - 1e-5) ** 2))

F32 = mybir.dt.float32
AOP = mybir.AluOpType
AF = mybir.ActivationFunctionType

LAST_RESULTS = None  # BassKernelResults of the most recent run (for profiling)
_CACHED = {}         # built Bass program, keyed by nothing (shapes are fixed)


def _fit_poly(w1, b1, wo, bo):
    """Degree-DEG monomial coeffs of P(d) = d^2*S(d) + bo*d on d in [0,1],
    S(d) = sum_k w1_k*wo_k*sigmoid(w1_k*d + b1_k).  Returns c[1..DEG]
    (c[0] is forced to 0 exactly)."""
    w1 = np.asarray(w1, np.float64).ravel()
    b1 = np.asarray(b1, np.float64).ravel()
    wo = np.asarray(wo, np.float64).ravel()
    bo = float(np.asarray(bo, np.float64).ravel()[0])
    c = w1 * wo
    d = np.linspace(0.0, 1.0, 20001)
    z = d[:, None] * w1[None, :] + b1[None, :]
    S = (c[None, :] / (1.0 + np.exp(-z))).sum(axis=1)
    P = d * d * S + bo * d
    cheb = np.polynomial.chebyshev.Chebyshev.fit(d, P, DEG, domain=[0.0, 1.0])
    coef = cheb.convert(kind=np.polynomial.Polynomial).coef
    coef = np.resize(coef, DEG + 1)
    coef[0] = 0.0
    return coef[1:].astype(np.float32)  # c_1 .. c_DEG


def _build_program(reps=1):
    nc = bass.Bass()
    rsj = nc.declare_dram_parameter("rsj", [3, 128, N], F32, isOutput=False)
    rsi = nc.declare_dram_parameter("rsi", [ROWS, 3], F32, isOutput=False)
    coefa = nc.declare_dram_parameter("coefa", [128, DEG], F32, isOutput=False)
    coefb = nc.declare_dram_parameter("coefb", [128, DEG], F32, isOutput=False)
    # Shape-bearing tag input: makes each reps-variant a distinct HLO module
    # (the NEFF compile cache keys on module fingerprint, which would
    # otherwise collide across reps since all real I/O shapes match).
    repstag = nc.declare_dram_parameter("repstag", [reps, 1], F32, isOutput=False)
    out = nc.declare_dram_parameter("out", [ROWS, 3], F32, isOutput=True)

    with TileContext(nc) as tc:
        with (
            tc.tile_pool(name="const", bufs=1) as cpool,
            tc.tile_pool(name="work", bufs=2) as wpool,
            tc.tile_pool(name="small", bufs=2) as spool,
        ):
            # Replicated j-coordinates, one [128, N] tile per coordinate.
            J = []
            for c in range(3):
                t = cpool.tile([128, N], F32, name=f"J{c}", tag=f"J{c}")
                nc.gpsimd.dma_start(out=t[:], in_=rsj[c])
                J.append(t)
            cA = cpool.tile([128, DEG], F32, tag="cA")
            nc.gpsimd.dma_start(out=cA[:], in_=coefa[:])
            cB = cpool.tile([128, DEG], F32, tag="cB")
            nc.gpsimd.dma_start(out=cB[:], in_=coefb[:])
            rtag = cpool.tile([1, 1], F32, tag="rtag")
            nc.gpsimd.dma_start(out=rtag[:], in_=repstag[reps - 1:reps, :])
            rsib = []
            for ib in range(NIB):
                t = cpool.tile([128, 3], F32, name=f"rsi{ib}", tag=f"rsi{ib}")
                nc.gpsimd.dma_start(out=t[:], in_=rsi[ib * 128:(ib + 1) * 128, :])
                rsib.append(t)

            for rep_ib in range(reps * NIB):
                ib = rep_ib % NIB
                sums = [spool.tile([128, NJT], F32, name=f"sums{c}", tag=f"sums{c}") for c in range(3)]
                for jt in range(NJT):
                    coef = cA if jt < NJT // 2 else cB
                    jsl = slice(jt * JT, (jt + 1) * JT)
                    # u = J - rs_i  in (-10, 10); minimum-image wrap via binary
                    # comparisons (no fp mod on this walrus; Sign() is unusable
                    # because Sign(0)=0 collapses |m| to 0 for pairs with
                    # u == +-5.0 exactly, which do occur among 50M pairs):
                    #   u1 = u - 10*(u >= 5);  m = u1 + 10*(u1 < -5)
                    # At |u|==5 exactly this yields |m|==5, where the force is
                    # exactly 0, so the (sign-ambiguous) boundary is harmless.
                    # Engine split (HW-measured): ACT instructions carry ~2us
                    # fixed cost each on this part, so ACT is reduced to the
                    # single mandatory Exp; everything else is DVE/GpSimd,
                    # balanced so neither engine dominates.
                    m = []
                    for c in range(3):
                        u = wpool.tile([128, JT], F32, name=f"u{c}", tag=f"u{c}")
                        nc.gpsimd.tensor_scalar(
                            u[:], J[c][:, jsl], rsib[ib][:, c:c + 1], None,
                            AOP.subtract)
                        ca = wpool.tile([128, JT], F32, name=f"ca{c}", tag=f"ca{c}")
                        nc.gpsimd.tensor_scalar(
                            ca[:], u[:], 5.0, 10.0, AOP.is_ge, AOP.mult)
                        E1 = nc.gpsimd if c < 2 else nc.vector
                        u1 = wpool.tile([128, JT], F32, name=f"u1{c}", tag=f"u1{c}")
                        E1.tensor_tensor(u1[:], u[:], ca[:], AOP.subtract)
                        cb = wpool.tile([128, JT], F32, name=f"cb{c}", tag=f"cb{c}")
                        nc.gpsimd.tensor_scalar(
                            cb[:], u1[:], -5.0, 10.0, AOP.is_lt, AOP.mult)
                        mc = wpool.tile([128, JT], F32, name=f"m{c}", tag=f"m{c}")
                        nc.vector.tensor_tensor(mc[:], u1[:], cb[:], AOP.add)
                        m.append(mc)
                    sq = []
                    for c in range(3):
                        s = wpool.tile([128, JT], F32, name=f"sq{c}", tag=f"sq{c}")
                        nc.vector.tensor_tensor(s[:], m[c][:], m[c][:], AOP.mult)
                        sq.append(s)
                    s3 = wpool.tile([128, JT], F32, tag="s3")
                    nc.vector.tensor_tensor(s3[:], sq[0][:], sq[1][:], AOP.add)
                    r2 = wpool.tile([128, JT], F32, tag="r2")
                    nc.vector.tensor_tensor(r2[:], s3[:], sq[2][:], AOP.add)
                    # g = clamp(1 - 0.04*r2, >= GMIN);  v = 1/g exactly on DVE
                    g = wpool.tile([128, JT], F32, tag="g")
                    nc.vector.tensor_scalar(
                        g[:], r2[:], -0.04, 1.0, AOP.mult, AOP.add)
                    gc = wpool.tile([128, JT], F32, tag="gc")
                    nc.vector.tensor_scalar(gc[:], g[:], GMIN, None, AOP.max)
                    v = wpool.tile([128, JT], F32, tag="v")
                    nc.vector.reciprocal(v[:], gc[:])
                    dcy = wpool.tile([128, JT], F32, tag="dcy")
                    nc.scalar.activation(dcy[:], v[:], AF.Exp, bias=1.0,
                                         scale=-1.0)
                    # Horner: F = (((c_D*d + c_{D-1})*d + ...)*d + c_1)*d
                    # via u_k = (u_{k+1} + c_k)*d, u_D = c_D*d; exact since c_0 = 0.
                    acc = wpool.tile([128, JT], F32, tag="acc0")
                    nc.vector.tensor_scalar(
                        acc[:], dcy[:], coef[:, DEG - 1:DEG], None, AOP.mult)
                    for k in range(DEG - 1, 0, -1):
                        nxt = wpool.tile([128, JT], F32, name=f"acc{(DEG - k) % 2}", tag=f"acc{(DEG - k) % 2}")
                        nc.vector.scalar_tensor_tensor(
                            nxt[:], acc[:], coef[:, k - 1:k], dcy[:],
                            AOP.add, AOP.mult)
                        acc = nxt
                    # Row-sums of F*m_c  (accumulated per j-tile into sums[c])
                    for c in range(3):
                        scratch = wpool.tile([128, JT], F32, tag="scratch")
                        nc.vector.scalar_tensor_tensor(
                            scratch[:], acc[:], 0.0, m[c][:],
                            AOP.bypass, AOP.mult,
                            accum_out=sums[c][:, jt:jt + 1])
                # Finalize block: out_rows = rs_i - sum(F*m)   (m = -true disp)
                res = spool.tile([128, 3], F32, tag="res")
                for c in range(3):
                    tot = spool.tile([128, 1], F32, name=f"tot{c}", tag=f"tot{c}")
                    nc.vector.tensor_reduce(
                        tot[:], sums[c][:], mybir.AxisListType.X, AOP.add)
                    nc.vector.tensor_scalar(
                        res[:, c:c + 1], tot[:], rsib[ib][:, c:c + 1], -1.0,
                        AOP.subtract, AOP.mult)
                nc.sync.dma_start(out=out[ib * 128:(ib + 1) * 128, :], in_=res[:])
    return nc


def _split_multi_waits(bir_json: bytes) -> bytes:
    """This walrus build rejects instructions carrying more than one sync
    wait ("Too many sync wait commands").  Hoist all-but-one wait of every
    instruction onto injected same-engine NoOps placed immediately before it
    (same blocking point on that engine's sequencer, so semantics are
    unchanged)."""
    import json as _json
    d = _json.loads(bir_json)
    for fn in d["functions"]:
        for blk in fn["blocks"]:
            new_insts = []
            for inst in blk["instructions"]:
                si = inst.get("sync_info")
                waits = (si or {}).get("on_wait") or []
                if len(waits) > 1:
                    for i, w in enumerate(waits[:-1]):
                        new_insts.append({
                            "debug": inst.get("debug", 0),
                            "engine": inst["engine"],
                            "ins": [],
                            "outs": [],
                            "name": f"{inst['name']}-w{i}",
                            "opcode": "NoOp",
                            "text_hint": "split_wait",
                            "sync_info": {"on_update": [], "on_wait": [w]},
                        })
                    si["on_wait"] = [waits[-1]]
                new_insts.append(inst)
            blk["instructions"] = new_insts
    return _json.dumps(d).encode()


def _get_program(reps=1):
    if reps not in _CACHED:
        nc = _build_program(reps)
        orig = nc.to_json_bytes
        nc.to_json_bytes = lambda: _split_multi_waits(orig())
        _CACHED[reps] = nc
    return _CACHED[reps]


def kernel(rs, same_w1, same_b1, same_wo, same_bo,
           diff_w1, diff_b1, diff_wo, diff_bo):
    global LAST_RESULTS
    rs = np.ascontiguousarray(np.asarray(rs, np.float32))
    coef_same = _fit_poly(same_w1, same_b1, same_wo, same_bo)
    coef_diff = _fit_poly(diff_w1, diff_b1, diff_wo, diff_bo)
    cs = np.ascontiguousarray(np.broadcast_to(coef_same[None, :], (128, DEG)))
    cd = np.ascontiguousarray(np.broadcast_to(coef_diff[None, :], (128, DEG)))

    rsj = np.ascontiguousarray(
        np.broadcast_to(rs.T[:, None, :], (3, 128, N)).astype(np.float32))

    in_maps = []
    for core in range(NCORES):
        up = (core * ROWS) < N_UP  # this core's rows are all one spin block
        in_maps.append({
            "rsj": rsj,
            "rsi": np.ascontiguousarray(rs[core * ROWS:(core + 1) * ROWS, :]),
            "coefa": cs if up else cd,   # coeffs for j < 2048
            "coefb": cd if up else cs,   # coeffs for j >= 2048
            "repstag": np.zeros((1, 1), np.float32),
        })

    nc = _get_program()
    LAST_RESULTS = run_bass_kernel_spmd(nc, in_maps, list(range(NCORES)))
    outs = [np.asarray(LAST_RESULTS.results[i]["out"]) for i in range(NCORES)]
    return np.concatenate(outs, axis=0).astype(np.float32)


# revision 16
# speedup vs baseline: 2.6441x; 2.6441x over previous
"""Trainium2 Bass kernel for the neural-backflow problem.

Problem (hardcoded shapes): rs (4096, 3) f32 in a periodic box L=10.
For every electron pair (i, j): minimum-image displacement d_ij, distance
r_ij, force f_ij = MLP_spin(r_ij) (1->32->1 swish MLP with compact-support
decay; "same" weights for same-spin pairs, "diff" for cross-spin), output
rs + sum_j f_ij * d_ij.

Key algebraic reduction used here: with z_k = decay*w1_k + b1_k,
  force = decay^2 * sum_k (w1_k*wo_k) * sigmoid(z_k) + bo*decay
which is a smooth scalar function P(decay) on decay in (0, 1].  We fit a
degree-10 polynomial (Chebyshev fit, monomial coeffs, P(0)=0 forced) to P at
kernel-call time from the actual weight values, so the device program is
input-independent: the MLP collapses to a Horner chain of
scalar_tensor_tensor ops on the Vector engine.

decay itself is computed exactly (not approximated):
  m   = ((rs_j - rs_i + 15) mod 10) - 5          (= -minimum-image disp)
  r2  = m_x^2 + m_y^2 + m_z^2                     (matches sqrt(r2+1e-15)^2)
  g   = clamp(1 - 0.04*(r2 + 1e-15), >= 1-(1-1e-5)^2)   (= 1 - xn^2)
  decay = exp(1 - 1/g),  with 1/g = exp(-ln g) on the ACT engine
  (ScalarE Reciprocal is banned; Ln/Exp/Square/Copy share one ACT table set)

Sharding: rows of the pair grid across 8 cores (512 rows each); rs is
replicated (pre-broadcast across 128 partitions host-side for the j-axis
tiles).  Row-sums are local per core; outputs are concatenated.
"""

import numpy as np

import concourse.bass as bass
import concourse.mybir as mybir
from concourse.tile import TileContext
from concourse.bass_utils import run_bass_kernel_spmd

L = 10.0
N = 4096
N_UP = 2048
NCORES = 8
ROWS = N // NCORES          # 512 rows per core
JT = 512                    # j-tile width
NJT = N // JT               # 8 j-tiles
NIB = ROWS // 128           # 4 i-blocks of 128 rows per core
DEG = 10                    # polynomial degree
GMIN = float(np.float32(1.0) - np.float32((1.0 - 1e-5) ** 2))

F32 = mybir.dt.float32
AOP = mybir.AluOpType
AF = mybir.ActivationFunctionType

LAST_RESULTS = None  # BassKernelResults of the most recent run (for profiling)
_CACHED = {}         # built Bass program, keyed by nothing (shapes are fixed)


def _fit_poly(w1, b1, wo, bo):
    """Degree-DEG monomial coeffs of P(d) = d^2*S(d) + bo*d on d in [0,1],
    S(d) = sum_k w1_k*wo_k*sigmoid(w1_k*d + b1_k).  Returns c[1..DEG]
    (c[0] is forced to 0 exactly)."""
    w1 = np.asarray(w1, np.float64).ravel()
    b1 = np.asarray(b1, np.float64).ravel()
    wo = np.asarray(wo, np.float64).ravel()
    bo = float(np.asarray(bo, np.float64).ravel()[0])
    c = w1 * wo
    d = np.linspace(0.0, 1.0, 20001)
    z = d[:, None] * w1[None, :] + b1[None, :]
    S = (c[None, :] / (1.0 + np.exp(-z))).sum(axis=1)
    P = d * d * S + bo * d
    cheb = np.polynomial.chebyshev.Chebyshev.fit(d, P, DEG, domain=[0.0, 1.0])
    coef = cheb.convert(kind=np.polynomial.Polynomial).coef
    coef = np.resize(coef, DEG + 1)
    coef[0] = 0.0
    return coef[1:].astype(np.float32)  # c_1 .. c_DEG


def _build_program(reps=1):
    nc = bass.Bass()
    rsj = nc.declare_dram_parameter("rsj", [3, 128, N], F32, isOutput=False)
    rsi = nc.declare_dram_parameter("rsi", [ROWS, 3], F32, isOutput=False)
    coefa = nc.declare_dram_parameter("coefa", [128, DEG], F32, isOutput=False)
    coefb = nc.declare_dram_parameter("coefb", [128, DEG], F32, isOutput=False)
    # Shape-bearing tag input: makes each reps-variant a distinct HLO module
    # (the NEFF compile cache keys on module fingerprint, which would
    # otherwise collide across reps since all real I/O shapes match).
    repstag = nc.declare_dram_parameter("repstag", [reps, 1], F32, isOutput=False)
    out = nc.declare_dram_parameter("out", [ROWS, 3], F32, isOutput=True)

    with TileContext(nc) as tc:
        with (
            tc.tile_pool(name="const", bufs=1) as cpool,
            tc.tile_pool(name="work", bufs=2) as wpool,
            tc.tile_pool(name="small", bufs=2) as spool,
        ):
            # Replicated j-coordinates, one [128, N] tile per coordinate.
            J = []
            for c in range(3):
                t = cpool.tile([128, N], F32, name=f"J{c}", tag=f"J{c}")
                nc.gpsimd.dma_start(out=t[:], in_=rsj[c])
                J.append(t)
            cA = cpool.tile([128, DEG], F32, tag="cA")
            nc.gpsimd.dma_start(out=cA[:], in_=coefa[:])
            cB = cpool.tile([128, DEG], F32, tag="cB")
            nc.gpsimd.dma_start(out=cB[:], in_=coefb[:])
            rtag = cpool.tile([1, 1], F32, tag="rtag")
            nc.gpsimd.dma_start(out=rtag[:], in_=repstag[reps - 1:reps, :])
            rsib = []
            for ib in range(NIB):
                t = cpool.tile([128, 3], F32, name=f"rsi{ib}", tag=f"rsi{ib}")
                nc.gpsimd.dma_start(out=t[:], in_=rsi[ib * 128:(ib + 1) * 128, :])
                rsib.append(t)

            for rep_ib in range(reps * NIB):
                ib = rep_ib % NIB
                sums = [spool.tile([128, NJT], F32, name=f"sums{c}", tag=f"sums{c}") for c in range(3)]
                for jt in range(NJT):
                    coef = cA if jt < NJT // 2 else cB
                    jsl = slice(jt * JT, (jt + 1) * JT)
                    # u = J - rs_i  in (-10, 10); minimum-image wrap via binary
                    # comparisons (no fp mod on this walrus; Sign() is unusable
                    # because Sign(0)=0 collapses |m| to 0 for pairs with
                    # u == +-5.0 exactly, which do occur among 50M pairs):
                    #   u1 = u - 10*(u >= 5);  m = u1 + 10*(u1 < -5)
                    # At |u|==5 exactly this yields |m|==5, where the force is
                    # exactly 0, so the (sign-ambiguous) boundary is harmless.
                    # Engine split (HW-measured): ACT instructions carry ~2us
                    # fixed cost each on this part, so ACT is reduced to the
                    # single mandatory Exp; everything else is DVE/GpSimd,
                    # balanced so neither engine dominates.
                    m = []
                    for c in range(3):
                        u = wpool.tile([128, JT], F32, name=f"u{c}", tag=f"u{c}")
                        nc.gpsimd.tensor_scalar(
                            u[:], J[c][:, jsl], rsib[ib][:, c:c + 1], None,
                            AOP.subtract)
                        ca = wpool.tile([128, JT], F32, name=f"ca{c}", tag=f"ca{c}")
                        nc.gpsimd.tensor_scalar(
                            ca[:], u[:], 5.0, 10.0, AOP.is_ge, AOP.mult)
                        E1 = nc.gpsimd if c < 2 else nc.vector
                        u1 = wpool.tile([128, JT], F32, name=f"u1{c}", tag=f"u1{c}")
                        E1.tensor_tensor(u1[:], u[:], ca[:], AOP.subtract)
                        cb = wpool.tile([128, JT], F32, name=f"cb{c}", tag=f"cb{c}")
                        nc.gpsimd.tensor_scalar(
                            cb[:], u1[:], -5.0, 10.0, AOP.is_lt, AOP.mult)
                        E2 = nc.gpsimd if c < 1 else nc.vector
                        mc = wpool.tile([128, JT], F32, name=f"m{c}", tag=f"m{c}")
                        E2.tensor_tensor(mc[:], u1[:], cb[:], AOP.add)
                        m.append(mc)
                    sq = []
                    for c in range(3):
                        s = wpool.tile([128, JT], F32, name=f"sq{c}", tag=f"sq{c}")
                        nc.vector.tensor_tensor(s[:], m[c][:], m[c][:], AOP.mult)
                        sq.append(s)
                    s3 = wpool.tile([128, JT], F32, tag="s3")
                    nc.vector.tensor_tensor(s3[:], sq[0][:], sq[1][:], AOP.add)
                    r2 = wpool.tile([128, JT], F32, tag="r2")
                    nc.vector.tensor_tensor(r2[:], s3[:], sq[2][:], AOP.add)
                    # g = clamp(1 - 0.04*r2, >= GMIN);  v = 1/g exactly on DVE
                    g = wpool.tile([128, JT], F32, tag="g")
                    nc.vector.tensor_scalar(
                        g[:], r2[:], -0.04, 1.0, AOP.mult, AOP.add)
                    gc = wpool.tile([128, JT], F32, tag="gc")
                    nc.vector.tensor_scalar(gc[:], g[:], GMIN, None, AOP.max)
                    v = wpool.tile([128, JT], F32, tag="v")
                    nc.vector.reciprocal(v[:], gc[:])
                    dcy = wpool.tile([128, JT], F32, tag="dcy")
                    nc.scalar.activation(dcy[:], v[:], AF.Exp, bias=1.0,
                                         scale=-1.0)
                    # Horner: F = (((c_D*d + c_{D-1})*d + ...)*d + c_1)*d
                    # via u_k = (u_{k+1} + c_k)*d, u_D = c_D*d; exact since c_0 = 0.
                    acc = wpool.tile([128, JT], F32, tag="acc0")
                    nc.vector.tensor_scalar(
                        acc[:], dcy[:], coef[:, DEG - 1:DEG], None, AOP.mult)
                    for k in range(DEG - 1, 0, -1):
                        nxt = wpool.tile([128, JT], F32, name=f"acc{(DEG - k) % 2}", tag=f"acc{(DEG - k) % 2}")
                        nc.vector.scalar_tensor_tensor(
                            nxt[:], acc[:], coef[:, k - 1:k], dcy[:],
                            AOP.add, AOP.mult)
                        acc = nxt
                    # Row-sums of F*m_c  (accumulated per j-tile into sums[c])
                    for c in range(3):
                        scratch = wpool.tile([128, JT], F32, tag="scratch")
                        nc.vector.scalar_tensor_tensor(
                            scratch[:], acc[:], 0.0, m[c][:],
                            AOP.bypass, AOP.mult,
                            accum_out=sums[c][:, jt:jt + 1])
                # Finalize block: out_rows = rs_i - sum(F*m)   (m = -true disp)
                res = spool.tile([128, 3], F32, tag="res")
                for c in range(3):
                    tot = spool.tile([128, 1], F32, name=f"tot{c}", tag=f"tot{c}")
                    nc.vector.tensor_reduce(
                        tot[:], sums[c][:], mybir.AxisListType.X, AOP.add)
                    nc.vector.tensor_scalar(
                        res[:, c:c + 1], tot[:], rsib[ib][:, c:c + 1], -1.0,
                        AOP.subtract, AOP.mult)
                nc.sync.dma_start(out=out[ib * 128:(ib + 1) * 128, :], in_=res[:])
    return nc


def _split_multi_waits(bir_json: bytes) -> bytes:
    """This walrus build rejects instructions carrying more than one sync
    wait ("Too many sync wait commands").  Hoist all-but-one wait of every
    instruction onto injected same-engine NoOps placed immediately before it
    (same blocking point on that engine's sequencer, so semantics are
    unchanged)."""
    import json as _json
    d = _json.loads(bir_json)
    for fn in d["functions"]:
        for blk in fn["blocks"]:
            new_insts = []
            for inst in blk["instructions"]:
                si = inst.get("sync_info")
                waits = (si or {}).get("on_wait") or []
                if len(waits) > 1:
                    for i, w in enumerate(waits[:-1]):
                        new_insts.append({
                            "debug": inst.get("debug", 0),
                            "engine": inst["engine"],
                            "ins": [],
                            "outs": [],
                            "name": f"{inst['name']}-w{i}",
                            "opcode": "NoOp",
                            "text_hint": "split_wait",
                            "sync_info": {"on_update": [], "on_wait": [w]},
                        })
                    si["on_wait"] = [waits[-1]]
                new_insts.append(inst)
            blk["instructions"] = new_insts
    return _json.dumps(d).encode()


def _get_program(reps=1):
    if reps not in _CACHED:
        nc = _build_program(reps)
        orig = nc.to_json_bytes
        nc.to_json_bytes = lambda: _split_multi_waits(orig())
        _CACHED[reps] = nc
    return _CACHED[reps]


def kernel(rs, same_w1, same_b1, same_wo, same_bo,
           diff_w1, diff_b1, diff_wo, diff_bo):
    global LAST_RESULTS
    rs = np.ascontiguousarray(np.asarray(rs, np.float32))
    coef_same = _fit_poly(same_w1, same_b1, same_wo, same_bo)
    coef_diff = _fit_poly(diff_w1, diff_b1, diff_wo, diff_bo)
    cs = np.ascontiguousarray(np.broadcast_to(coef_same[None, :], (128, DEG)))
    cd = np.ascontiguousarray(np.broadcast_to(coef_diff[None, :], (128, DEG)))

    rsj = np.ascontiguousarray(
        np.broadcast_to(rs.T[:, None, :], (3, 128, N)).astype(np.float32))

    in_maps = []
    for core in range(NCORES):
        up = (core * ROWS) < N_UP  # this core's rows are all one spin block
        in_maps.append({
            "rsj": rsj,
            "rsi": np.ascontiguousarray(rs[core * ROWS:(core + 1) * ROWS, :]),
            "coefa": cs if up else cd,   # coeffs for j < 2048
            "coefb": cd if up else cs,   # coeffs for j >= 2048
            "repstag": np.zeros((1, 1), np.float32),
        })

    nc = _get_program()
    LAST_RESULTS = run_bass_kernel_spmd(nc, in_maps, list(range(NCORES)))
    outs = [np.asarray(LAST_RESULTS.results[i]["out"]) for i in range(NCORES)]
    return np.concatenate(outs, axis=0).astype(np.float32)
